# revision 5
# baseline (speedup 1.0000x reference)
"""Trainium2 Bass kernel for nn_CSBrain (per-region electrode conv, kernels 1/3/5).

Strategy:
  - Data-parallel over batch: 8 cores x 2 batches each.
  - Host marshals x into an f-major (transposed) fp16 layout with a per-region
    circular halo (2 electrodes each side) and an appended ones-row, so the
    bias can ride the matmul as an extra contraction row.
  - Weights are host-packed into a single (region, 201, 500) fp16 "Wcat":
    columns grouped by output-electrode offset delta in {+2,+1,0,-1,-2} so each
    (electrode, f-half) stationary tile needs only <=5 matmuls, each writing a
    contiguous column range of the per-electrode PSUM accumulator.
  - Device: per (batch, region): load x tiles, for each extended electrode slot
    run matmuls with the x tile stationary (lhsT) and Wcat columns moving,
    accumulating out[t, d] tiles in PSUM (fp32); drain pairs of finished
    electrodes through DVE/ACT copies (cast fp16) and DMA out.
  - Host unscrambles the (b, t, c, d) fp16 device output to (B, C, T, D) fp32.
"""

import sys

if "/opt/trn_rl_repo" not in sys.path:
    sys.path.insert(0, "/opt/trn_rl_repo")

import numpy as np

REGION_SIZES = [12, 14, 12, 14, 12]
REGION_STARTS = [0, 12, 26, 38, 52]
B, C, T, F = 16, 64, 128, 200
DIM_OUT = 200
N_CORES = 8
B_LOC = B // N_CORES  # 2
HALO = 2
SLOTS = [ne + 2 * HALO for ne in REGION_SIZES]  # 16,18,16,18,16
COL_OFFS = np.cumsum([0] + [s * T for s in SLOTS]).tolist()  # per-region col offset
NCOLS = COL_OFFS[-1]  # 84*128 = 10752
KLO = 128  # f rows 0:128 in the lo tile
KHI = F - KLO + 1  # 73 = f rows 128:200 plus the ones/bias row

# Wcat column ranges per delta group (delta = out_electrode - in_electrode)
GCOLS = {2: (0, 50), 1: (50, 150), 0: (150, 350), -1: (350, 450), -2: (450, 500)}
# matching output column ranges in the per-electrode accumulator
OCOLS = {2: (150, 200), 1: (100, 200), 0: (0, 200), -1: (100, 200), -2: (150, 200)}
DELTAS = (2, 1, 0, -1, -2)

_CACHE = {}


def _build_nc(loop_reps=1):
    import concourse.tile as tile
    from concourse import bacc, mybir
    import concourse.bass as bass
    from contextlib import ExitStack

    f16 = mybir.dt.float16
    f32 = mybir.dt.float32

    nc = bacc.Bacc(
        "TRN2",
        target_bir_lowering=False,
        debug=False,
        num_devices=N_CORES,
    )
    xin = nc.dram_tensor("xin", [B_LOC, 201, NCOLS], f16, kind="ExternalInput").ap()
    wcat = nc.dram_tensor("wcat", [5, 201, 500], f16, kind="ExternalInput").ap()
    out = nc.dram_tensor(
        "out", [B_LOC, T, C * DIM_OUT], f16, kind="ExternalOutput"
    ).ap()

    with tile.TileContext(nc) as tc:
        with (
            tc.tile_pool(name="w", bufs=1) as wpool,
            tc.tile_pool(name="x", bufs=2) as xpool,
            tc.tile_pool(name="ps", bufs=8, space=bass.MemorySpace.PSUM) as pspool,
            tc.tile_pool(name="st", bufs=4) as stpool,
        ):
            wlo, whi = [], []
            for r in range(5):
                tl = wpool.tile([KLO, 500], f16, tag=f"wlo{r}")
                nc.sync.dma_start(tl[:], wcat[r, 0:KLO, :])
                th = wpool.tile([KHI, 500], f16, tag=f"whi{r}")
                nc.sync.dma_start(th[:], wcat[r, KLO : KLO + KHI, :])
                wlo.append(tl)
                whi.append(th)

            loop_ctx = ExitStack()
            if loop_reps > 1:
                loop_ctx.enter_context(
                    tc.For_i(
                        0,
                        loop_reps,
                        1,
                        hint_engines=(mybir.EngineType.PE,),
                    )
                )
            for bl in range(B_LOC):
                for r in range(5):
                    ne = REGION_SIZES[r]
                    slots = SLOTS[r]
                    off = COL_OFFS[r]
                    ncols = slots * T
                    XL = xpool.tile([KLO, ncols], f16, tag="xl")
                    nc.sync.dma_start(XL[:], xin[bl, 0:KLO, off : off + ncols])
                    XH = xpool.tile([KHI, ncols], f16, tag="xh")
                    nc.sync.dma_start(
                        XH[:], xin[bl, KLO : KLO + KHI, off : off + ncols]
                    )
                    acc = {}
                    for s in range(slots):
                        for half in (0, 1):
                            if half == 0:
                                xt = XL[:, s * T : (s + 1) * T]
                                w = wlo[r]
                            else:
                                xt = XH[:, s * T : (s + 1) * T]
                                w = whi[r]
                            for delta in DELTAS:
                                e = s - HALO + delta
                                if not (0 <= e < ne):
                                    continue
                                if half == 0 and delta == 2:
                                    acc[e] = pspool.tile([T, DIM_OUT], f32, tag="acc", name="acc")
                                g0, g1 = GCOLS[delta]
                                o0, o1 = OCOLS[delta]
                                nc.tensor.matmul(
                                    acc[e][:, o0:o1],
                                    xt,
                                    w[:, g0:g1],
                                    start=(half == 0 and delta == 2),
                                    stop=(half == 1 and delta == -2),
                                )
                        edone = s - 2 * HALO
                        if 0 <= edone < ne and edone % 2 == 1:
                            stage = stpool.tile([T, 2 * DIM_OUT], f16, tag="stage")
                            nc.vector.tensor_copy(
                                stage[:, 0:DIM_OUT], acc[edone - 1][:]
                            )
                            nc.scalar.copy(
                                stage[:, DIM_OUT : 2 * DIM_OUT], acc[edone][:]
                            )
                            cabs = REGION_STARTS[r] + edone - 1
                            nc.sync.dma_start(
                                out[
                                    bl,
                                    :,
                                    cabs * DIM_OUT : (cabs + 2) * DIM_OUT,
                                ],
                                stage[:],
                            )
                            del acc[edone - 1], acc[edone]
            loop_ctx.close()

    nc.compile()
    return nc


def _get_nc(loop_reps=1):
    key = ("nc", loop_reps)
    if key not in _CACHE:
        _CACHE[key] = _build_nc(loop_reps)
    return _CACHE[key]


def _marshal_x(x):
    """x (B, C, T, F) fp32 -> (N_CORES, B_LOC, 201, NCOLS) fp16, f-major with
    halo and ones-row."""
    xin = np.empty((B, 201, NCOLS), np.float16)
    for r in range(5):
        ne = REGION_SIZES[r]
        s0 = REGION_STARTS[r]
        off = COL_OFFS[r]
        idx = (np.arange(SLOTS[r]) - HALO) % ne
        xr = x[:, s0 + idx, :, :]  # (B, S, T, F)
        arr = np.transpose(xr, (0, 3, 1, 2)).reshape(B, F, SLOTS[r] * T)
        xin[:, 0:F, off : off + SLOTS[r] * T] = arr.astype(np.float16)
    xin[:, F, :] = np.float16(1.0)
    return xin.reshape(N_CORES, B_LOC, 201, NCOLS)


def _marshal_w(W1, b1, W3, b3, W5, b5):
    """Pack weights into (5, 201, 500) fp16 Wcat (f rows 0:200, bias row 200)."""
    wcat = np.zeros((5, 201, 500), np.float32)

    def put(col, W, j):
        d = W.shape[1]
        wcat[:, 0:F, col : col + d] = np.transpose(W[:, :, :, j], (0, 2, 1))
        return col + d

    # delta=+2 : k5 j0
    put(0, W5, 0)
    # delta=+1 : k3 j0, k5 j1
    put(50, W3, 0)
    put(100, W5, 1)
    # delta=0 : k1 j0, k3 j1, k5 j2 (center taps -> carry bias)
    put(150, W1, 0)
    put(250, W3, 1)
    put(300, W5, 2)
    wcat[:, F, 150:250] = b1
    wcat[:, F, 250:300] = b3
    wcat[:, F, 300:350] = b5
    # delta=-1 : k3 j2, k5 j3
    put(350, W3, 2)
    put(400, W5, 3)
    # delta=-2 : k5 j4
    put(450, W5, 4)
    return wcat.astype(np.float16)


def _unmarshal(outs):
    """outs: list of N_CORES arrays (B_LOC, T, C*DIM_OUT) fp16 -> (B,C,T,D) fp32."""
    dev = np.stack(outs).reshape(B, T, C, DIM_OUT)
    return np.ascontiguousarray(dev.transpose(0, 2, 1, 3)).astype(np.float32)


def _run(in_maps, **kwargs):
    from concourse.bass_utils import run_bass_kernel_spmd

    nc = _get_nc()
    return run_bass_kernel_spmd(nc, in_maps, core_ids=list(range(N_CORES)), **kwargs)


def make_in_maps(x, W1, b1, W3, b3, W5, b5):
    xin = _marshal_x(np.asarray(x, dtype=np.float32))
    wcat = _marshal_w(
        np.asarray(W1), np.asarray(b1), np.asarray(W3), np.asarray(b3),
        np.asarray(W5), np.asarray(b5),
    )
    return [{"xin": xin[m], "wcat": wcat} for m in range(N_CORES)]


def kernel(x, W1, b1, W3, b3, W5, b5):
    in_maps = make_in_maps(x, W1, b1, W3, b3, W5, b5)
    res = _run(in_maps)
    return _unmarshal([res.results[m]["out"] for m in range(N_CORES)])


# revision 34
# speedup vs baseline: 5.8310x; 5.8310x over previous
"""Trainium2 Bass kernel for nn_CSBrain (per-region electrode conv, kernels 1/3/5).

Strategy:
  - Data-parallel over batch: 8 cores x 2 batches each.
  - Host marshals x into an f-major (transposed) fp16 layout (b, f_aug, c*T)
    with an appended ones-row so the bias can ride the matmul as an extra
    contraction row. Circular electrode indexing is done with modulo column
    addressing on-device (no data duplication).
  - Weights are host-packed into a single (region, 201, 500) fp16 "Wcat":
    columns grouped by output-electrode offset delta in {+2,+1,0,-1,-2} so each
    (electrode, f-half) stationary tile needs only <=5 matmuls, each writing a
    contiguous column range of the per-electrode PSUM accumulator.
  - Device: per batch: two big x loads (f rows 0:128 and 128:201); per region,
    for each extended electrode slot run matmuls with the x tile stationary
    (lhsT) and Wcat columns moving, accumulating out[t, d] tiles in PSUM
    (fp32); drain finished electrodes through DVE/ACT copies (cast fp16) into
    a region staging tile; one output store per region.
  - Host unscrambles the (b, t, c, d) fp16 device output to (B, C, T, D) fp32.
"""

import sys

if "/opt/trn_rl_repo" not in sys.path:
    sys.path.insert(0, "/opt/trn_rl_repo")

import numpy as np

REGION_SIZES = [12, 14, 12, 14, 12]
REGION_STARTS = [0, 12, 26, 38, 52]
B, C, T, F = 16, 64, 128, 200
DIM_OUT = 200
N_CORES = 8
B_LOC = B // N_CORES  # 2
HALO = 2
NCOLS = C * T  # 8192
KLO = 128  # f rows 0:128 in the lo tile
KHI = F - KLO + 1  # 73 = f rows 128:200 plus the ones/bias row

# Wcat column ranges per delta group (delta = out_electrode - in_electrode)
GCOLS = {2: (0, 50), 1: (50, 150), 0: (150, 350), -1: (350, 450), -2: (450, 500)}
# matching output column ranges in the per-electrode accumulator
OCOLS = {2: (150, 200), 1: (100, 200), 0: (0, 200), -1: (100, 200), -2: (150, 200)}
DELTAS = (2, 1, 0, -1, -2)

_CACHE = {}


def _dedup_ldweights(nc):
    """Drop consecutive InstLdweights that reload the identical stationary AP.

    The Rust lowering emits one Ldweights per Matmult; matmuls sharing a
    stationary tile reload it redundantly (~107ns each on PE). Walrus supports
    one Ldweights feeding several non-self-loading Matmults, so dropping the
    duplicates is safe as long as any semaphore waits they carry are migrated
    to the next instruction.
    """
    import concourse.mybir as mybir

    removed = 0
    for fn in nc.m.functions:
        for blk in fn.blocks:
            insts = blk.instructions
            last_sig = None
            drop = []
            for idx, inst in enumerate(insts):
                tn = type(inst).__name__
                si = inst.sync_info
                if tn == "InstLdweights":
                    sig = (
                        str(inst.ins[0]),
                        str(inst.tile_size),
                        str(inst.tile_position),
                        str(inst.perf_mode),
                        str(inst.is_transpose),
                    )
                    has_upd = si is not None and len(si.on_update) > 0
                    if sig == last_sig and not has_upd and idx + 1 < len(insts):
                        if si is not None and len(si.on_wait) > 0:
                            nxt = insts[idx + 1]
                            nsi = nxt.sync_info
                            if nsi is None:
                                nxt.sync_info = mybir.SyncInfo(
                                    on_wait=list(si.on_wait), on_update=[]
                                )
                            else:
                                nsi.on_wait = list(nsi.on_wait) + list(si.on_wait)
                        drop.append(idx)
                        removed += 1
                    else:
                        last_sig = sig
                elif tn in ("InstMatmult", "InstNop", "InstEventSemaphore"):
                    pass
                else:
                    last_sig = None
            for idx in reversed(drop):
                del insts[idx]
    return removed


def _build_nc(loop_reps=1, variant="full", unroll=False):
    import concourse.tile as tile
    from concourse import bacc, mybir
    import concourse.bass as bass
    from contextlib import ExitStack

    do_mm = variant in ("full", "mm", "fullnostore")
    do_drain = variant in ("full", "fullnostore")
    do_store = variant == "full"

    f16 = mybir.dt.float16
    f32 = mybir.dt.float32

    nc = bacc.Bacc(
        "TRN2",
        target_bir_lowering=False,
        debug=False,
        num_devices=N_CORES,
    )
    xin = nc.dram_tensor("xin", [B_LOC, 201, NCOLS], f16, kind="ExternalInput").ap()
    wcat = nc.dram_tensor("wcat", [5, 201, 500], f16, kind="ExternalInput").ap()
    out = nc.dram_tensor(
        "out", [B_LOC, T, C * DIM_OUT], f16, kind="ExternalOutput"
    ).ap()

    # load chunk split (column ranges): region 0, regions 1-2, regions 3-4
    CHUNKS = [(0, 12 * T), (12 * T, 38 * T), (38 * T, NCOLS)]

    with tile.TileContext(nc) as tc:
        with (
            tc.tile_pool(name="w", bufs=1) as wpool,
            tc.tile_pool(name="x", bufs=1) as xpool,
            tc.tile_pool(name="ps", bufs=8, space=bass.MemorySpace.PSUM) as pspool,
            tc.tile_pool(name="st", bufs=3) as stpool,
        ):
            # persistent x tiles (manual double-buffer across the two batches);
            # row 72 of each hi tile holds the ones vector, loaded once
            xls = [
                xpool.tile([KLO, NCOLS], f16, tag=f"xl{bl}", name="xls")
                for bl in range(B_LOC)
            ]
            xhs = [
                xpool.tile([KHI, NCOLS], f16, tag=f"xh{bl}", name="xhs")
                for bl in range(B_LOC)
            ]
            def _load_x(bl, chunks):
                for c0, c1 in chunks:
                    nc.sync.dma_start(
                        xls[bl][:, c0:c1], xin[bl, 0:KLO, c0:c1]
                    )
                    nc.scalar.dma_start(
                        xhs[bl][0:72, c0:c1], xin[bl, KLO:F, c0:c1]
                    )

            # startup ordering: first chunk of batch 0 leads both HWDGE rings,
            # the ones rows ride the sync ring right behind it, weights stream
            # on the gpsimd (SWDGE) ring in region order.
            # startup: XL-A leads sync, XH-A leads scalar; region-0 hi weights
            # and the ones rows ride sync right behind XL-A; lo weights and the
            # remaining regions' weights stream on the gpsimd (SWDGE) ring.
            _load_x(0, CHUNKS[:1])
            wlo = [
                wpool.tile([KLO, 500], f16, tag=f"wlo{r}", name="wlo_t")
                for r in range(5)
            ]
            whi = [
                wpool.tile([KHI, 500], f16, tag=f"whi{r}", name="whi_t")
                for r in range(5)
            ]
            nc.sync.dma_start(whi[0][0:72, :], wcat[0, KLO:F, :])
            nc.sync.dma_start(whi[0][72:73, :], wcat[0, F : F + 1, :])
            for bl in range(B_LOC):
                nc.sync.dma_start(xhs[bl][72:73, :], xin[0, 200:201, :])
            nc.gpsimd.dma_start(wlo[0][:], wcat[0, 0:KLO, :])
            for r in range(1, 5):
                nc.gpsimd.dma_start(wlo[r][:], wcat[r, 0:KLO, :])
                nc.gpsimd.dma_start(whi[r][0:72, :], wcat[r, KLO:F, :])
                nc.gpsimd.dma_start(whi[r][72:73, :], wcat[r, F : F + 1, :])

            if variant == "mm":
                _load_x(0, CHUNKS[1:])
                _load_x(1, CHUNKS)

            loop_ctx = ExitStack()
            if loop_reps > 1 and not unroll:
                loop_ctx.enter_context(
                    tc.For_i(
                        0,
                        loop_reps,
                        1,
                        hint_engines=(mybir.EngineType.PE,),
                    )
                )
            for _rep in range(loop_reps if unroll else 1):
              for bl in range(B_LOC):
                XL, XH = xls[bl], xhs[bl]
                if variant != "mm":
                    _load_x(bl, CHUNKS[1:] if bl == 0 else CHUNKS)
                for r in range(5):
                    ne = REGION_SIZES[r]
                    slots = ne + 2 * HALO
                    stage = None
                    if do_drain or do_store:
                        stage = stpool.tile(
                            [T, ne * DIM_OUT], f16, tag="stage", name="stage"
                        )
                    acc = {}
                    for s in range(slots if do_mm else 0):
                        cphys = (s - HALO) % ne
                        col0 = (REGION_STARTS[r] + cphys) * T
                        for half in (0, 1):
                            if half == 0:
                                xt = XL[:, col0 : col0 + T]
                                w = wlo[r]
                            else:
                                xt = XH[:, col0 : col0 + T]
                                w = whi[r]
                            for delta in DELTAS:
                                e = s - HALO + delta
                                if not (0 <= e < ne):
                                    continue
                                if half == 0 and delta == 2:
                                    acc[e] = pspool.tile(
                                        [T, DIM_OUT], f32, tag="acc", name="acc"
                                    )
                                g0, g1 = GCOLS[delta]
                                o0, o1 = OCOLS[delta]
                                nc.tensor.matmul(
                                    acc[e][:, o0:o1],
                                    xt,
                                    w[:, g0:g1],
                                    start=(half == 0 and delta == 2),
                                    stop=(half == 1 and delta == -2),
                                )
                        edone = s - 2 * HALO
                        if do_drain and 0 <= edone < ne and edone % 2 == 1:
                            c0 = (edone - 1) * DIM_OUT
                            nc.vector.tensor_copy(
                                stage[:, c0 : c0 + DIM_OUT], acc[edone - 1][:]
                            )
                            nc.scalar.copy(
                                stage[:, c0 + DIM_OUT : c0 + 2 * DIM_OUT],
                                acc[edone][:],
                            )
                            del acc[edone - 1], acc[edone]
                    if do_store:
                        o0 = REGION_STARTS[r] * DIM_OUT
                        h = (ne // 2) * DIM_OUT
                        nc.gpsimd.dma_start(
                            out[bl, :, o0 : o0 + h], stage[:, 0:h]
                        )
                        nc.sync.dma_start(
                            out[bl, :, o0 + h : o0 + ne * DIM_OUT],
                            stage[:, h : ne * DIM_OUT],
                        )
            loop_ctx.close()

    _dedup_ldweights(nc)
    nc.compile()
    return nc


def _get_nc(loop_reps=1, variant="full", unroll=False):
    key = ("nc", loop_reps, variant, unroll)
    if key not in _CACHE:
        _CACHE[key] = _build_nc(loop_reps, variant, unroll)
    return _CACHE[key]


def _marshal_x(x):
    """x (B, C, T, F) fp32 -> (N_CORES, B_LOC, 201, C*T) fp16 f-major + ones."""
    xin = np.empty((B, 201, NCOLS), np.float16)
    xin[:, 0:F, :] = (
        np.transpose(x, (0, 3, 1, 2)).reshape(B, F, NCOLS).astype(np.float16)
    )
    xin[:, F, :] = np.float16(1.0)
    return xin.reshape(N_CORES, B_LOC, 201, NCOLS)


def _marshal_w(W1, b1, W3, b3, W5, b5):
    """Pack weights into (5, 201, 500) fp16 Wcat (f rows 0:200, bias row 200)."""
    wcat = np.zeros((5, 201, 500), np.float32)

    def put(col, W, j):
        d = W.shape[1]
        wcat[:, 0:F, col : col + d] = np.transpose(W[:, :, :, j], (0, 2, 1))
        return col + d

    # delta=+2 : k5 j0
    put(0, W5, 0)
    # delta=+1 : k3 j0, k5 j1
    put(50, W3, 0)
    put(100, W5, 1)
    # delta=0 : k1 j0, k3 j1, k5 j2 (center taps -> carry bias)
    put(150, W1, 0)
    put(250, W3, 1)
    put(300, W5, 2)
    wcat[:, F, 150:250] = b1
    wcat[:, F, 250:300] = b3
    wcat[:, F, 300:350] = b5
    # delta=-1 : k3 j2, k5 j3
    put(350, W3, 2)
    put(400, W5, 3)
    # delta=-2 : k5 j4
    put(450, W5, 4)
    return wcat.astype(np.float16)


def _unmarshal(outs):
    """outs: list of N_CORES arrays (B_LOC, T, C*DIM_OUT) fp16 -> (B,C,T,D) fp32."""
    dev = np.stack(outs).reshape(B, T, C, DIM_OUT)
    return np.ascontiguousarray(dev.transpose(0, 2, 1, 3)).astype(np.float32)


def _run(in_maps, **kwargs):
    from concourse.bass_utils import run_bass_kernel_spmd

    nc = _get_nc()
    return run_bass_kernel_spmd(nc, in_maps, core_ids=list(range(N_CORES)), **kwargs)


def make_in_maps(x, W1, b1, W3, b3, W5, b5):
    xin = _marshal_x(np.asarray(x, dtype=np.float32))
    wcat = _marshal_w(
        np.asarray(W1), np.asarray(b1), np.asarray(W3), np.asarray(b3),
        np.asarray(W5), np.asarray(b5),
    )
    return [{"xin": xin[m], "wcat": wcat} for m in range(N_CORES)]


def kernel(x, W1, b1, W3, b3, W5, b5):
    in_maps = make_in_maps(x, W1, b1, W3, b3, W5, b5)
    res = _run(in_maps)
    return _unmarshal([res.results[m]["out"] for m in range(N_CORES)])
